# revision 1
# baseline (speedup 1.0000x reference)
"""ChildSum TreeLSTM on 8 Trainium2 NeuronCores.

Sharding: the input graph is a forest (every non-top-level node has exactly
one parent). Subtrees are closed under the level-synchronous recurrence, so
we partition the roots across the 8 cores (greedy balance by subtree size)
and each core computes its subtrees with zero cross-core communication.

Within a core, each level's nodes are renumbered in parent-sorted order so
that the children of level-l parents are exactly the level-(l-1) nodes in
slot order: child state reads become contiguous SBUF slices (no gather).

SPMD uniformity: one Bass program runs on all 8 cores, so all shapes are
padded to the cross-core max per level, and the set of (edge-chunk,
parent-chunk) segment-sum matmuls is the union across cores; a core with no
overlap for a pair contributes an all-zero one-hot.
"""

import math
import os

import numpy as np

P = 128
NCORES = 8


# ---------------------------------------------------------------- host planning
def _ceil_to(x, m):
    return max(m, ((int(x) + m - 1) // m) * m)


def build_plan(features, node_order, adjacency_list, edge_order, num_levels):
    N = int(features.shape[0])
    L = int(num_levels)
    lvl = np.asarray(node_order, np.int64)
    parent_g = np.asarray(adjacency_list[:, 0], np.int64)
    child_g = np.asarray(adjacency_list[:, 1], np.int64)

    par_of = np.full(N, -1, np.int64)
    par_of[child_g] = parent_g

    # root of each node (L-1 pointer jumps)
    r = np.arange(N, dtype=np.int64)
    for _ in range(L - 1):
        p = par_of[r]
        r = np.where(p >= 0, p, r)

    root_ids = np.flatnonzero(lvl == L - 1)
    ridx = np.searchsorted(root_ids, r)
    sizes = np.bincount(ridx, minlength=len(root_ids))
    order_desc = np.argsort(-sizes, kind="stable")
    loads = np.zeros(NCORES, np.int64)
    assign = np.zeros(len(root_ids), np.int64)
    for i in order_desc:
        b = int(np.argmin(loads))
        loads[b] += sizes[i]
        assign[i] = b
    core_of = assign[ridx]

    # per-core per-level node orders; level-l order = children of level-(l+1)
    # parents in parent-slot order (so edges at level l+1 are contiguous)
    orders = [[None] * L for _ in range(NCORES)]
    slot_of = np.full(N, -1, np.int64)
    counts = np.zeros((NCORES, L), np.int64)
    for c in range(NCORES):
        sel = core_of == c
        top = np.flatnonzero(sel & (lvl == L - 1))
        orders[c][L - 1] = top
        slot_of[top] = np.arange(len(top))
        counts[c][L - 1] = len(top)
        for l in range(L - 2, -1, -1):
            nl = np.flatnonzero(sel & (lvl == l))
            key = slot_of[par_of[nl]]
            o = np.argsort(key, kind="stable")
            nlo = nl[o]
            orders[c][l] = nlo
            slot_of[nlo] = np.arange(len(nlo))
            counts[c][l] = len(nlo)

    PN = [int(_ceil_to(counts[:, l].max(), P)) for l in range(L)]
    Lbase = np.concatenate([[0], np.cumsum(PN)]).astype(np.int64)
    NT = int(Lbase[-1])
    NCH = NT // P

    # edge data: level l >= 1 has PE_l = PN_{l-1} (padded) edges; edge e's
    # child slot is e (identity!), parent slot is slot_of[parent(child)]
    PE = [0] + [PN[l - 1] for l in range(1, L)]
    ECbase = np.concatenate([[0], np.cumsum([PE[l] // P for l in range(L)])]).astype(
        np.int64
    )
    NEC = int(ECbase[-1])

    feat = np.asarray(features, np.int64)
    featidx = np.zeros((NCORES, NT), np.int32)
    maskv = np.zeros((NCORES, NT), np.float32)
    gids = np.full((NCORES, NT), -1, np.int64)
    pslot = np.zeros((NCORES, sum(PE)), np.int64)  # per-level concat of parent slots
    PEbase = np.concatenate([[0], np.cumsum(PE)]).astype(np.int64)

    for c in range(NCORES):
        for l in range(L):
            n = int(counts[c][l])
            b = int(Lbase[l])
            ids = orders[c][l]
            featidx[c, b : b + n] = feat[ids].astype(np.int32)
            maskv[c, b : b + n] = 1.0
            gids[c, b : b + n] = ids
            if l >= 1:
                eb = int(PEbase[l])
                ne = int(counts[c][l - 1])  # one edge per level-(l-1) node
                ch_ids = orders[c][l - 1]
                ps = slot_of[par_of[ch_ids]]
                assert np.all(np.diff(ps) >= 0)
                pslot[c, eb : eb + ne] = ps
                pslot[c, eb + ne : eb + PE[l]] = min(int(counts[c][l]), PN[l] - 1)

    # wf gather row index per edge (int32 rows into wf_dram [NT, 128])
    wfidx = np.zeros((NCORES, NEC * P), np.int32)
    for c in range(NCORES):
        for l in range(1, L):
            eb, pe = int(PEbase[l]), PE[l]
            ob = int(ECbase[l]) * P
            wfidx[c, ob : ob + pe] = (Lbase[l] + pslot[c, eb : eb + pe]).astype(
                np.int32
            )

    # (ec, pc) pair union per level + rel vectors
    pairs = [[] for _ in range(L)]  # per level: list of (ec_local, pc_local)
    rel_cols = []  # per pair: (l, ec, pc)
    for l in range(1, L):
        eb = int(PEbase[l])
        necs = PE[l] // P
        for ec in range(necs):
            pcs = set()
            for c in range(NCORES):
                sl = pslot[c, eb + ec * P : eb + (ec + 1) * P]
                pcs.update(np.unique(sl // P).tolist())
            for pc in sorted(pcs):
                pairs[l].append((ec, int(pc)))
                rel_cols.append((l, ec, int(pc)))
    NPAIR = len(rel_cols)
    rel = np.full((NCORES, NPAIR, P), -1.0, np.float32)
    for j, (l, ec, pc) in enumerate(rel_cols):
        eb = int(PEbase[l])
        for c in range(NCORES):
            sl = pslot[c, eb + ec * P : eb + (ec + 1) * P] - pc * P
            ok = (sl >= 0) & (sl < P)
            rel[c, j] = np.where(ok, sl, -1.0).astype(np.float32)

    # chunks that are pads in every core (skip compute, just write zeros)
    allpad_chunk = np.ones(NCH, bool)
    for c in range(NCORES):
        m = maskv[c].reshape(NCH, P)
        allpad_chunk &= ~m.any(axis=1)
    # chunks needing a mask (some core has a pad row in them)
    need_mask = np.zeros(NCH, bool)
    for c in range(NCORES):
        m = maskv[c].reshape(NCH, P)
        need_mask |= m.any(axis=1) & ~m.all(axis=1)
    # any core-pad row in a computed chunk => mask it
    for c in range(NCORES):
        m = maskv[c].reshape(NCH, P)
        need_mask |= (~allpad_chunk) & ~m.all(axis=1)

    return dict(
        N=N,
        L=L,
        PN=PN,
        PE=PE,
        Lbase=Lbase,
        PEbase=PEbase,
        ECbase=ECbase,
        NT=NT,
        NCH=NCH,
        NEC=NEC,
        NPAIR=NPAIR,
        pairs=pairs,
        rel_cols=rel_cols,
        featidx=featidx,
        wfidx=wfidx,
        rel=rel,
        maskv=maskv,
        gids=gids,
        counts=counts,
        allpad_chunk=allpad_chunk,
        need_mask=need_mask,
    )


# ---------------------------------------------------------------- bass builder
def build_bass(plan, vocab, has_bias, mm_dtype="float32", wf_param=False):
    import concourse.bacc as bacc
    import concourse.bass as _bass
    import concourse.tile as tile
    from concourse import mybir
    from concourse.masks import make_identity

    L = plan["L"]
    PN, PE = plan["PN"], plan["PE"]
    Lbase, ECbase = plan["Lbase"], plan["ECbase"]
    NT, NCH, NEC, NPAIR = plan["NT"], plan["NCH"], plan["NEC"], plan["NPAIR"]
    pairs_by_level = plan["pairs"]
    allpad = plan["allpad_chunk"]
    need_mask = plan["need_mask"]

    f32 = mybir.dt.float32
    i32 = mybir.dt.int32
    mmdt = getattr(mybir.dt, mm_dtype)
    AF = mybir.ActivationFunctionType
    OP = mybir.AluOpType

    nc = bacc.Bacc()
    emb_d = nc.declare_dram_parameter("emb", [vocab, P], f32, isOutput=False)
    wcat_d = nc.declare_dram_parameter("wcat", [P, 512], f32, isOutput=False)
    uiou_d = nc.declare_dram_parameter("uiou", [P, 384], f32, isOutput=False)
    uf_d = nc.declare_dram_parameter("uf", [P, P], f32, isOutput=False)
    featidx_d = nc.declare_dram_parameter("featidx", [P, NCH], i32, isOutput=False)
    wfidx_d = nc.declare_dram_parameter("wfidx", [P, NEC], i32, isOutput=False)
    if NPAIR:
        rel_d = nc.declare_dram_parameter("rel", [P, NPAIR], f32, isOutput=False)
    mask_d = nc.declare_dram_parameter("maskv", [P, NCH], f32, isOutput=False)
    if has_bias:
        bias_d = nc.declare_dram_parameter("bias", [P, 512], f32, isOutput=False)
    outh_d = nc.declare_dram_parameter("out_h", [NT, P], f32, isOutput=True)
    outc_d = nc.declare_dram_parameter("out_c", [NT, P], f32, isOutput=True)
    if wf_param:
        wf_dram = nc.declare_dram_parameter("wf_host", [NT, P], f32, isOutput=False)
    else:
        wf_dram = nc.dram_tensor("wf_dram", [NT, P], f32)

    def mm(x):  # view an f32 AP as the matmul dtype
        return x if mm_dtype == "float32" else x.bitcast(mmdt)

    with tile.TileContext(nc) as tc:
        with (
            tc.tile_pool(name="const", bufs=1) as cpool,
            tc.tile_pool(name="state", bufs=1) as spool,
            tc.tile_pool(name="work", bufs=3) as wpool,
            tc.tile_pool(name="psx", bufs=2, space="PSUM") as psx,
            tc.tile_pool(name="pst", bufs=1, space="PSUM") as pst,
            tc.tile_pool(name="psz", bufs=1, space="PSUM") as psz,
            tc.tile_pool(name="pseg", bufs=2, space="PSUM") as pseg,
        ):
            # constants
            w_sb = cpool.tile([P, 512], f32, tag="w")
            nc.sync.dma_start(w_sb[:], wcat_d[:])
            uiou_sb = cpool.tile([P, 384], f32, tag="uiou")
            nc.sync.dma_start(uiou_sb[:], uiou_d[:])
            uf_sb = cpool.tile([P, P], f32, tag="uf")
            nc.sync.dma_start(uf_sb[:], uf_d[:])
            fidx_sb = cpool.tile([P, NCH], i32, tag="fidx")
            nc.sync.dma_start(fidx_sb[:], featidx_d[:])
            wfidx_sb = cpool.tile([P, NEC], i32, tag="wfidx")
            nc.sync.dma_start(wfidx_sb[:], wfidx_d[:])
            if NPAIR:
                rel_sb = cpool.tile([P, NPAIR], f32, tag="rel")
                nc.sync.dma_start(rel_sb[:], rel_d[:])
            mask_sb = cpool.tile([P, NCH], f32, tag="mask")
            nc.sync.dma_start(mask_sb[:], mask_d[:])
            if has_bias:
                bias_sb = cpool.tile([P, 512], f32, tag="bias")
                nc.sync.dma_start(bias_sb[:], bias_d[:])
            ident = cpool.tile([P, P], f32, tag="ident")
            make_identity(nc, ident[:])
            iota_i = cpool.tile([P, P], i32, tag="iotai")
            nc.gpsimd.iota(iota_i[:], [[1, P]], channel_multiplier=0)
            iota_f = cpool.tile([P, P], f32, tag="iotaf")
            nc.vector.tensor_copy(iota_f[:], iota_i[:])

            h_all = spool.tile([P, NT], f32, tag="h")
            c_all = spool.tile([P, NT], f32, tag="c")
            wiou_cols = 3 * max(PN[l] for l in range(1, L)) if L > 1 else 384
            wiou_lvl = spool.tile([P, wiou_cols], f32, tag="wiou")

            def xproj(l, j):
                """gather + transpose + x @ W for parent chunk j of level l.
                Returns psum tile [P, 512] (cols 0:384 iou, 384:512 wf)."""
                g = int(Lbase[l]) // P + j
                xt = wpool.tile([P, P], f32, tag="xt")
                nc.gpsimd.indirect_dma_start(
                    out=xt[:],
                    out_offset=None,
                    in_=emb_d[:],
                    in_offset=_bass.IndirectOffsetOnAxis(
                        ap=fidx_sb[:, g : g + 1], axis=0
                    ),
                )
                xT_ps = pst.tile([P, P], f32, tag="pst")
                nc.tensor.transpose(xT_ps[:], xt[:], ident[:])
                xT = wpool.tile([P, P], f32, tag="xT")
                nc.scalar.copy(xT[:], xT_ps[:])
                wcols = 384 if l == 0 else 512
                ps = psx.tile([P, 512], f32, tag="psx")
                nc.tensor.matmul(
                    ps[:, :wcols], mm(xT[:]), mm(w_sb[:, :wcols]), start=True, stop=True
                )
                return ps, g

            def gates_store(l, j, g, i_t, o_t, u_t, cs_ap):
                """c = (i*u)*mask + cs ; h = o*tanh(c)*mask ; DMA out."""
                c_sl = c_all[:, g * P : (g + 1) * P]
                h_sl = h_all[:, g * P : (g + 1) * P]
                msk = need_mask[g]
                tmp = wpool.tile([P, P], f32, tag="tmp")
                if msk:
                    nc.vector.scalar_tensor_tensor(
                        out=tmp[:],
                        in0=i_t,
                        scalar=mask_sb[:, g : g + 1],
                        in1=u_t,
                        op0=OP.mult,
                        op1=OP.mult,
                    )
                else:
                    nc.vector.tensor_tensor(tmp[:], i_t, u_t, op=OP.mult)
                if cs_ap is None:
                    nc.vector.tensor_copy(c_sl, tmp[:])
                else:
                    nc.vector.tensor_tensor(c_sl, tmp[:], cs_ap, op=OP.add)
                t_t = wpool.tile([P, P], f32, tag="tt")
                nc.scalar.activation(t_t[:], c_sl, AF.Tanh)
                if msk:
                    nc.vector.scalar_tensor_tensor(
                        out=h_sl,
                        in0=o_t,
                        scalar=mask_sb[:, g : g + 1],
                        in1=t_t[:],
                        op0=OP.mult,
                        op1=OP.mult,
                    )
                else:
                    nc.vector.tensor_tensor(h_sl, o_t, t_t[:], op=OP.mult)
                r0 = g * P
                nc.sync.dma_start(outh_d[r0 : r0 + P, :], h_sl)
                nc.sync.dma_start(outc_d[r0 : r0 + P, :], c_sl)

            def pad_chunk(g):
                c_sl = c_all[:, g * P : (g + 1) * P]
                h_sl = h_all[:, g * P : (g + 1) * P]
                nc.vector.memset(c_sl, 0.0)
                nc.vector.memset(h_sl, 0.0)
                r0 = g * P
                nc.sync.dma_start(outh_d[r0 : r0 + P, :], h_sl)
                nc.sync.dma_start(outc_d[r0 : r0 + P, :], c_sl)

            # ---------------- level 0
            for j in range(PN[0] // P):
                g = j
                if allpad[g]:
                    pad_chunk(g)
                    continue
                ps, g = xproj(0, j)
                if has_bias:
                    zb = wpool.tile([P, 384], f32, tag="zb")
                    nc.vector.tensor_tensor(
                        zb[:], ps[:, :384], bias_sb[:, :384], op=OP.add
                    )
                    src = zb[:]
                else:
                    src = ps[:, :384]
                io_t = wpool.tile([P, 256], f32, tag="iot")
                nc.scalar.activation(io_t[:], src[:, 0:256], AF.Sigmoid)
                u_t = wpool.tile([P, P], f32, tag="ut")
                nc.scalar.activation(u_t[:], src[:, 256:384], AF.Tanh)
                gates_store(
                    0, j, g, io_t[:, 0:128], io_t[:, 128:256], u_t[:], None
                )

            # ---------------- levels 1..L-1
            for l in range(1, L):
                nch = PN[l] // P
                base_g = int(Lbase[l]) // P
                # phase A: x-proj for this level's parents -> wiou (sbuf) + wf (dram)
                for j in range(nch):
                    g = base_g + j
                    if allpad[g]:
                        continue
                    ps, g = xproj(l, j)
                    wi = wiou_lvl[:, j * 384 : (j + 1) * 384]
                    if has_bias:
                        nc.vector.tensor_tensor(
                            wi, ps[:, :384], bias_sb[:, :384], op=OP.add
                        )
                    else:
                        nc.scalar.copy(wi, ps[:, :384])
                    if not wf_param:
                        wfst = wpool.tile([P, P], f32, tag="wfst")
                        if has_bias:
                            nc.vector.tensor_tensor(
                                wfst[:], ps[:, 384:512], bias_sb[:, 384:512], op=OP.add
                            )
                        else:
                            nc.vector.tensor_copy(wfst[:], ps[:, 384:512])
                        r0 = g * P
                        nc.sync.dma_start(wf_dram[r0 : r0 + P, :], wfst[:])

                # phase B1: per edge chunk, f = sigmoid(h_ch @ U_f + wf[par]);
                # overwrite c_all child slice with f*c (children are dead after
                # their output DMA, so in-place is safe)
                lv_pairs = pairs_by_level[l]
                relcol_of = {
                    (ll, ec, pc): i
                    for i, (ll, ec, pc) in enumerate(plan["rel_cols"])
                }
                necs = PE[l] // P
                prev_base_g = int(Lbase[l - 1]) // P
                for ec in range(necs):
                    gch = prev_base_g + ec
                    if allpad[gch]:
                        continue  # fc stays 0 (slice was memset by pad_chunk)
                    ch = h_all[:, gch * P : (gch + 1) * P]
                    cc = c_all[:, gch * P : (gch + 1) * P]
                    chT_ps = pst.tile([P, P], f32, tag="pst", name=f"chT_{l}_{ec}")
                    nc.tensor.transpose(chT_ps[:], ch, ident[:])
                    chT = wpool.tile([P, P], f32, tag="chT", name=f"chTs_{l}_{ec}")
                    nc.scalar.copy(chT[:], chT_ps[:])
                    z_ps = psz.tile([P, P], f32, tag="psz", name=f"z_{l}_{ec}")
                    nc.tensor.matmul(
                        z_ps[:], mm(chT[:]), mm(uf_sb[:]), start=True, stop=True
                    )
                    wfe = wpool.tile([P, P], f32, tag="wfe", name=f"wfe_{l}_{ec}")
                    ecg = int(ECbase[l]) + ec
                    nc.gpsimd.indirect_dma_start(
                        out=wfe[:],
                        out_offset=None,
                        in_=wf_dram[:],
                        in_offset=_bass.IndirectOffsetOnAxis(
                            ap=wfidx_sb[:, ecg : ecg + 1], axis=0
                        ),
                    )
                    zf = wpool.tile([P, P], f32, tag="zf", name=f"zf_{l}_{ec}")
                    nc.vector.tensor_tensor(zf[:], z_ps[:], wfe[:], op=OP.add)
                    f_t = wpool.tile([P, P], f32, tag="ft", name=f"f_{l}_{ec}")
                    nc.scalar.activation(f_t[:], zf[:], AF.Sigmoid)
                    nc.vector.tensor_tensor(cc, f_t[:], cc, op=OP.mult)

                # phase B2+C: parent-chunk-major segment sums — exactly one
                # PSUM accumulation open at a time (pseg bufs=2 double-buffers)
                by_pc = {}
                for ec, pc in lv_pairs:
                    by_pc.setdefault(pc, []).append(ec)
                for pc in range(nch):
                    g = base_g + pc
                    ecs = [
                        e for e in by_pc.get(pc, []) if not allpad[prev_base_g + e]
                    ]
                    if allpad[g] or not ecs:
                        pad_chunk(g)
                        continue
                    # two PSUM tiles: one open accumulation group per bank
                    segA = pseg.tile([P, P], f32, tag="segA", name=f"segA_{l}_{pc}")
                    segB = pseg.tile([P, P], f32, tag="segB", name=f"segB_{l}_{pc}")
                    for k, ec in enumerate(ecs):
                        gch = prev_base_g + ec
                        ch = h_all[:, gch * P : (gch + 1) * P]
                        fc = c_all[:, gch * P : (gch + 1) * P]
                        rcol = relcol_of[(l, ec, pc)]
                        oh = wpool.tile([P, P], f32, tag="oh", name=f"oh_{l}_{pc}_{ec}")
                        nc.gpsimd.tensor_scalar(
                            oh[:],
                            iota_f[:],
                            rel_sb[:, rcol : rcol + 1],
                            None,
                            op0=OP.is_equal,
                        )
                        fst = k == 0
                        lst = k == len(ecs) - 1
                        # h_sumT accumulated directly: lhsT=ch gives [H, parent]
                        nc.tensor.matmul(
                            segA[:], mm(ch), mm(oh[:]), start=fst, stop=lst
                        )
                        nc.tensor.matmul(
                            segB[:], mm(oh[:]), mm(fc), start=fst, stop=lst
                        )
                    hscs = wpool.tile([P, 256], f32, tag="hscs", name=f"hscs_{l}_{pc}")
                    nc.vector.tensor_copy(hscs[:, 0:128], segA[:])
                    nc.vector.tensor_copy(hscs[:, 128:256], segB[:])
                    iou_ps = psx.tile([P, 512], f32, tag="psx", name=f"iou_{l}_{pc}")
                    nc.tensor.matmul(
                        iou_ps[:, :384],
                        mm(hscs[:, 0:128]),
                        mm(uiou_sb[:]),
                        start=True,
                        stop=True,
                    )
                    iou = wpool.tile([P, 384], f32, tag="iou", name=f"ioub_{l}_{pc}")
                    nc.vector.tensor_tensor(
                        iou[:],
                        iou_ps[:, :384],
                        wiou_lvl[:, pc * 384 : (pc + 1) * 384],
                        op=OP.add,
                    )
                    io_t = wpool.tile([P, 256], f32, tag="iot", name=f"io_{l}_{pc}")
                    nc.scalar.activation(io_t[:], iou[:, 0:256], AF.Sigmoid)
                    u_t = wpool.tile([P, P], f32, tag="ut", name=f"u_{l}_{pc}")
                    nc.scalar.activation(u_t[:], iou[:, 256:384], AF.Tanh)
                    gates_store(
                        l, pc, g, io_t[:, 0:128], io_t[:, 128:256], u_t[:],
                        hscs[:, 128:256],
                    )

    nc.finalize()
    return nc


# ---------------------------------------------------------------- entry point
def kernel(
    features,
    node_order,
    adjacency_list,
    edge_order,
    emb,
    W_iou,
    b_iou,
    U_iou,
    W_f,
    b_f,
    U_f,
    num_levels,
):
    from concourse.bass_utils import run_bass_kernel_spmd

    features = np.asarray(features)
    node_order = np.asarray(node_order)
    adjacency_list = np.asarray(adjacency_list)
    edge_order = np.asarray(edge_order)
    emb = np.ascontiguousarray(np.asarray(emb, np.float32))
    W_iou = np.asarray(W_iou, np.float32)
    b_iou = np.asarray(b_iou, np.float32)
    U_iou = np.ascontiguousarray(np.asarray(U_iou, np.float32))
    W_f = np.asarray(W_f, np.float32)
    b_f = np.asarray(b_f, np.float32)
    U_f = np.ascontiguousarray(np.asarray(U_f, np.float32))
    L = int(num_levels)

    plan = build_plan(features, node_order, adjacency_list, edge_order, L)

    wcat = np.ascontiguousarray(np.concatenate([W_iou, W_f], axis=1))
    bias = np.concatenate([b_iou, b_f])
    has_bias = bool(np.any(bias != 0.0))

    mm_dtype = os.environ.get("TREELSTM_MM_DTYPE", "float32")
    wf_param = os.environ.get("TREELSTM_WF_PARAM", "0") == "1"
    nc = build_bass(
        plan, int(emb.shape[0]), has_bias, mm_dtype=mm_dtype, wf_param=wf_param
    )

    NCH, NEC, NPAIR = plan["NCH"], plan["NEC"], plan["NPAIR"]
    in_maps = []
    for c in range(NCORES):
        m = {
            "emb": emb,
            "wcat": wcat,
            "uiou": U_iou,
            "uf": U_f,
            "featidx": np.ascontiguousarray(
                plan["featidx"][c].reshape(NCH, P).T
            ),
            "wfidx": np.ascontiguousarray(plan["wfidx"][c].reshape(NEC, P).T),
            "maskv": np.ascontiguousarray(plan["maskv"][c].reshape(NCH, P).T),
        }
        if NPAIR:
            m["rel"] = np.ascontiguousarray(plan["rel"][c].T)
        if has_bias:
            m["bias"] = np.ascontiguousarray(
                np.broadcast_to(bias[None, :], (P, 512))
            )
        if wf_param:
            m["wf_host"] = np.ascontiguousarray(
                emb[plan["featidx"][c]] @ W_f + b_f
            )
        in_maps.append(m)

    trace = os.environ.get("TREELSTM_TRACE", "0") == "1"
    res = run_bass_kernel_spmd(nc, in_maps, list(range(NCORES)), trace=trace)
    if trace and res.exec_time_ns is not None:
        print(f"HW exec time: {res.exec_time_ns} ns", flush=True)

    N = plan["N"]
    H = P
    h_full = np.zeros((N, H), np.float32)
    c_full = np.zeros((N, H), np.float32)
    for c in range(NCORES):
        gid = plan["gids"][c]
        rows = np.flatnonzero(gid >= 0)
        h_full[gid[rows]] = res.results[c]["out_h"][rows]
        c_full[gid[rows]] = res.results[c]["out_c"][rows]
    return h_full, c_full



# revision 11
# speedup vs baseline: 1.9685x; 1.9685x over previous
"""ChildSum TreeLSTM on 8 Trainium2 NeuronCores.

Sharding: the input graph is a forest (every non-top-level node has exactly
one parent). Subtrees are closed under the level-synchronous recurrence, so
we partition the roots across the 8 cores (greedy balance by subtree size)
and each core computes its subtrees with zero cross-core communication.

Within a core, each level's nodes are renumbered in parent-sorted order so
that the children of level-l parents are exactly the level-(l-1) nodes in
slot order: child state reads become contiguous SBUF slices (no gather).

SPMD uniformity: one Bass program runs on all 8 cores, so all shapes are
padded to the cross-core max per level, and the set of (edge-chunk,
parent-chunk) segment-sum matmuls is the union across cores; a core with no
overlap for a pair contributes an all-zero one-hot.

v2: all-bf16 datapath. One-hot segment-sum matrices are precomputed on the
host and streamed to SBUF (no gpsimd tensor_scalar). The per-edge forget
input wf[parent] is produced by accumulating ohT.T @ wf_chunk into the same
PSUM group as h_child @ U_f (no DRAM roundtrip, no indirect gather). All
transposes run on the DMA xbar (dma_start_transpose, batched per 8 chunks).
Embedding gathers are batched indirect DMAs. h|c state is interleaved per
chunk so each chunk stores with a single output DMA.
"""

import math
import os

import numpy as np

P = 128
NCORES = 8


# ---------------------------------------------------------------- host planning
def _ceil_to(x, m):
    return max(m, ((int(x) + m - 1) // m) * m)


def build_plan(features, node_order, adjacency_list, edge_order, num_levels):
    N = int(features.shape[0])
    L = int(num_levels)
    lvl = np.asarray(node_order, np.int64)
    parent_g = np.asarray(adjacency_list[:, 0], np.int64)
    child_g = np.asarray(adjacency_list[:, 1], np.int64)

    par_of = np.full(N, -1, np.int64)
    par_of[child_g] = parent_g

    # root of each node (L-1 pointer jumps)
    r = np.arange(N, dtype=np.int64)
    for _ in range(L - 1):
        p = par_of[r]
        r = np.where(p >= 0, p, r)

    root_ids = np.flatnonzero(lvl == L - 1)
    ridx = np.searchsorted(root_ids, r)
    sizes = np.bincount(ridx, minlength=len(root_ids))
    order_desc = np.argsort(-sizes, kind="stable")
    loads = np.zeros(NCORES, np.int64)
    assign = np.zeros(len(root_ids), np.int64)
    for i in order_desc:
        b = int(np.argmin(loads))
        loads[b] += sizes[i]
        assign[i] = b
    core_of = assign[ridx]

    # per-core per-level node orders; level-l order = children of level-(l+1)
    # parents in parent-slot order (so edges at level l+1 are contiguous)
    orders = [[None] * L for _ in range(NCORES)]
    slot_of = np.full(N, -1, np.int64)
    counts = np.zeros((NCORES, L), np.int64)
    for c in range(NCORES):
        sel = core_of == c
        top = np.flatnonzero(sel & (lvl == L - 1))
        orders[c][L - 1] = top
        slot_of[top] = np.arange(len(top))
        counts[c][L - 1] = len(top)
        for l in range(L - 2, -1, -1):
            nl = np.flatnonzero(sel & (lvl == l))
            key = slot_of[par_of[nl]]
            o = np.argsort(key, kind="stable")
            nlo = nl[o]
            orders[c][l] = nlo
            slot_of[nlo] = np.arange(len(nlo))
            counts[c][l] = len(nlo)

    PN = [int(_ceil_to(counts[:, l].max(), P)) for l in range(L)]
    Lbase = np.concatenate([[0], np.cumsum(PN)]).astype(np.int64)
    NT = int(Lbase[-1])
    NCH = NT // P

    # edge data: level l >= 1 has PE_l = PN_{l-1} (padded) edges; edge e's
    # child slot is e (identity!), parent slot is slot_of[parent(child)]
    PE = [0] + [PN[l - 1] for l in range(1, L)]

    feat = np.asarray(features, np.int64)
    featidx = np.zeros((NCORES, NT), np.int32)
    maskv = np.zeros((NCORES, NT), np.float32)
    gids = np.full((NCORES, NT), -1, np.int64)
    pslot = np.zeros((NCORES, sum(PE)), np.int64)  # per-level concat of parent slots
    PEbase = np.concatenate([[0], np.cumsum(PE)]).astype(np.int64)

    for c in range(NCORES):
        for l in range(L):
            n = int(counts[c][l])
            b = int(Lbase[l])
            ids = orders[c][l]
            featidx[c, b : b + n] = feat[ids].astype(np.int32)
            maskv[c, b : b + n] = 1.0
            gids[c, b : b + n] = ids
            if l >= 1:
                eb = int(PEbase[l])
                ne = int(counts[c][l - 1])  # one edge per level-(l-1) node
                ch_ids = orders[c][l - 1]
                ps = slot_of[par_of[ch_ids]]
                assert np.all(np.diff(ps) >= 0)
                pslot[c, eb : eb + ne] = ps
                pslot[c, eb + ne : eb + PE[l]] = min(int(counts[c][l]), PN[l] - 1)

    # (ec, pc) pair union per level + rel vectors
    pairs = [[] for _ in range(L)]  # per level: list of (ec_local, pc_local)
    rel_cols = []  # per pair: (l, ec, pc)
    for l in range(1, L):
        eb = int(PEbase[l])
        necs = PE[l] // P
        for ec in range(necs):
            pcs = set()
            for c in range(NCORES):
                sl = pslot[c, eb + ec * P : eb + (ec + 1) * P]
                pcs.update(np.unique(sl // P).tolist())
            for pc in sorted(pcs):
                pairs[l].append((ec, int(pc)))
                rel_cols.append((l, ec, int(pc)))
    NPAIR = len(rel_cols)
    rel = np.full((NCORES, NPAIR, P), -1.0, np.float32)
    for j, (l, ec, pc) in enumerate(rel_cols):
        eb = int(PEbase[l])
        for c in range(NCORES):
            sl = pslot[c, eb + ec * P : eb + (ec + 1) * P] - pc * P
            ok = (sl >= 0) & (sl < P)
            rel[c, j] = np.where(ok, sl, -1.0).astype(np.float32)

    # chunks that are pads in every core (skip compute, just write zeros)
    allpad_chunk = np.ones(NCH, bool)
    for c in range(NCORES):
        m = maskv[c].reshape(NCH, P)
        allpad_chunk &= ~m.any(axis=1)
    # chunks needing a mask (some core has a pad row in them)
    need_mask = np.zeros(NCH, bool)
    for c in range(NCORES):
        m = maskv[c].reshape(NCH, P)
        need_mask |= m.any(axis=1) & ~m.all(axis=1)
    for c in range(NCORES):
        m = maskv[c].reshape(NCH, P)
        need_mask |= (~allpad_chunk) & ~m.all(axis=1)

    return dict(
        N=N,
        L=L,
        PN=PN,
        PE=PE,
        Lbase=Lbase,
        PEbase=PEbase,
        NT=NT,
        NCH=NCH,
        NPAIR=NPAIR,
        pairs=pairs,
        rel_cols=rel_cols,
        featidx=featidx,
        rel=rel,
        maskv=maskv,
        gids=gids,
        counts=counts,
        allpad_chunk=allpad_chunk,
        need_mask=need_mask,
    )


# ---------------------------------------------------------------- bass builder
def build_bass(plan, vocab, has_bias, gb=8):
    import concourse.bacc as bacc
    import concourse.bass as _bass
    import concourse.tile as tile
    from concourse import mybir

    L = plan["L"]
    PN, PE = plan["PN"], plan["PE"]
    Lbase = plan["Lbase"]
    NT, NCH, NPAIR = plan["NT"], plan["NCH"], plan["NPAIR"]
    pairs_by_level = plan["pairs"]
    rel_cols = plan["rel_cols"]
    allpad = plan["allpad_chunk"]
    need_mask = plan["need_mask"]

    f32 = mybir.dt.float32
    bf = mybir.dt.bfloat16
    i32 = mybir.dt.int32
    AF = mybir.ActivationFunctionType
    OP = mybir.AluOpType

    max_nch_hi = max((PN[l] // P for l in range(1, L)), default=1)

    nc = bacc.Bacc()
    emb_d = nc.declare_dram_parameter("emb", [vocab, P], bf, isOutput=False)
    wcat_d = nc.declare_dram_parameter("wcat", [P, 512], bf, isOutput=False)
    uiou_d = nc.declare_dram_parameter("uiou", [P, 384], bf, isOutput=False)
    uf_d = nc.declare_dram_parameter("uf", [P, P], bf, isOutput=False)
    featidx_d = nc.declare_dram_parameter("featidx", [P, NCH], i32, isOutput=False)
    if NPAIR:
        oh_d = nc.declare_dram_parameter("ohcat", [P, NPAIR * P], bf, isOutput=False)
        oht_d = nc.declare_dram_parameter("ohtcat", [P, NPAIR * P], bf, isOutput=False)
    mask_d = nc.declare_dram_parameter("maskv", [P, NCH], bf, isOutput=False)
    if has_bias:
        bias_d = nc.declare_dram_parameter("bias", [P, 512], bf, isOutput=False)
    outhc_d = nc.declare_dram_parameter("outhc", [NT, 2 * P], bf, isOutput=True)

    with tile.TileContext(nc) as tc:
        with (
            tc.tile_pool(name="const", bufs=1) as cpool,
            tc.tile_pool(name="state", bufs=1) as spool,
            tc.tile_pool(name="lvl", bufs=2) as lpool,
            tc.tile_pool(name="lvl1", bufs=1) as lpool1,
            tc.tile_pool(name="work", bufs=3) as wpool,
            tc.tile_pool(name="bigw", bufs=2) as wpool2,
            tc.tile_pool(name="psio", bufs=2, space="PSUM") as psio,
            tc.tile_pool(name="pszw", bufs=2, space="PSUM") as pszw,
            tc.tile_pool(name="psA", bufs=2, space="PSUM") as psA,
            tc.tile_pool(name="psB", bufs=2, space="PSUM") as psB,
        ):
            # constants
            w_sb = cpool.tile([P, 512], bf, tag="w")
            nc.sync.dma_start(w_sb[:], wcat_d[:])
            uiou_sb = cpool.tile([P, 384], bf, tag="uiou")
            nc.sync.dma_start(uiou_sb[:], uiou_d[:])
            uf_sb = cpool.tile([P, P], bf, tag="uf")
            nc.sync.dma_start(uf_sb[:], uf_d[:])
            fidx_sb = cpool.tile([P, NCH], i32, tag="fidx")
            nc.sync.dma_start(fidx_sb[:], featidx_d[:])
            mask_sb = cpool.tile([P, NCH], bf, tag="mask")
            nc.sync.dma_start(mask_sb[:], mask_d[:])
            if NPAIR:
                oh_sb = cpool.tile([P, NPAIR * P], bf, tag="oh")
                nc.sync.dma_start(oh_sb[:], oh_d[:])
                oht_sb = cpool.tile([P, NPAIR * P], bf, tag="oht")
                nc.sync.dma_start(oht_sb[:], oht_d[:])
            if has_bias:
                bias_sb = cpool.tile([P, 512], bf, tag="bias")
                nc.sync.dma_start(bias_sb[:], bias_d[:])

            # interleaved state: chunk g = cols [2gP, 2gP+128) c | [2gP+128, 2gP+256) h
            hc_all = spool.tile([P, 2 * NT], bf, tag="hc")

            def cslice(g):
                return hc_all[:, 2 * g * P : 2 * g * P + P]

            def hslice(g):
                return hc_all[:, 2 * g * P + P : 2 * g * P + 2 * P]

            def out_dma(g):
                r0 = g * P
                nc.sync.dma_start(
                    outhc_d[r0 : r0 + P, :], hc_all[:, 2 * g * P : 2 * (g + 1) * P]
                )

            def pad_chunk(g):
                nc.vector.memset(hc_all[:, 2 * g * P : 2 * (g + 1) * P], 0.0)
                out_dma(g)

            def gates(g, iou_ap, cs_ap):
                """i,o,u from iou_ap ([P,384] psum); c/h into hc_all; store."""
                c_sl, h_sl = cslice(g), hslice(g)
                msk = need_mask[g]
                io_t = wpool.tile([P, 256], bf, tag="iot", name=f"io_{g}")
                nc.scalar.activation(io_t[:], iou_ap[:, 0:256], AF.Sigmoid)
                u_t = wpool.tile([P, P], bf, tag="ut", name=f"u_{g}")
                nc.scalar.activation(u_t[:], iou_ap[:, 256:384], AF.Tanh)
                tmp = wpool.tile([P, P], bf, tag="tmp", name=f"tmp_{g}")
                if msk:
                    nc.vector.scalar_tensor_tensor(
                        out=tmp[:],
                        in0=io_t[:, 0:128],
                        scalar=mask_sb[:, g : g + 1],
                        in1=u_t[:],
                        op0=OP.mult,
                        op1=OP.mult,
                    )
                else:
                    nc.vector.tensor_tensor(tmp[:], io_t[:, 0:128], u_t[:], op=OP.mult)
                if cs_ap is None:
                    nc.vector.tensor_copy(c_sl, tmp[:])
                else:
                    nc.vector.tensor_tensor(c_sl, tmp[:], cs_ap, op=OP.add)
                t_t = wpool.tile([P, P], bf, tag="tt", name=f"t_{g}")
                nc.scalar.activation(t_t[:], c_sl, AF.Tanh)
                if msk:
                    nc.vector.scalar_tensor_tensor(
                        out=h_sl,
                        in0=io_t[:, 128:256],
                        scalar=mask_sb[:, g : g + 1],
                        in1=t_t[:],
                        op0=OP.mult,
                        op1=OP.mult,
                    )
                else:
                    nc.vector.tensor_tensor(h_sl, io_t[:, 128:256], t_t[:], op=OP.mult)
                out_dma(g)

            relcol_of = {(ll, ec, pc): i for i, (ll, ec, pc) in enumerate(rel_cols)}

            def gather_chunk(dst_ap, gcol):
                nc.gpsimd.indirect_dma_start(
                    out=dst_ap,
                    out_offset=None,
                    in_=emb_d[:],
                    in_offset=_bass.IndirectOffsetOnAxis(
                        ap=fidx_sb[:, gcol : gcol + 1], axis=0
                    ),
                )

            # ---------------- level 0: batched gather+transpose, per-chunk mm
            nch0 = PN[0] // P
            for b0 in range(0, nch0, gb):
                w0 = min(gb, nch0 - b0)
                xg = wpool2.tile([P, gb * P], bf, tag="xg", name=f"xg0_{b0}")
                for jj in range(w0):
                    gather_chunk(xg[:, jj * P : (jj + 1) * P], b0 + jj)
                xT = wpool2.tile([P, gb, P], bf, tag="xT0", name=f"xT0_{b0}")
                nc.sync.dma_start_transpose(xT[:, :w0, :], xg[:, : w0 * P])
                for jj in range(w0):
                    g = b0 + jj
                    if allpad[g]:
                        pad_chunk(g)
                        continue
                    iou_ps = psio.tile([P, 384], f32, tag="psio", name=f"iou0_{g}")
                    nc.tensor.matmul(
                        iou_ps[:], xT[:, jj, :], w_sb[:, 0:384], start=True, stop=True
                    )
                    if has_bias:
                        iou_sb = wpool.tile([P, 384], bf, tag="ioub", name=f"zb_{g}")
                        nc.vector.tensor_tensor(
                            iou_sb[:], iou_ps[:], bias_sb[:, 0:384], op=OP.add
                        )
                        gates(g, iou_sb, None)
                    else:
                        gates(g, iou_ps, None)

            # ---------------- levels 1..L-1
            for l in range(1, L):
                nch = PN[l] // P
                base_g = int(Lbase[l]) // P
                prev_base_g = int(Lbase[l - 1]) // P
                necs = PE[l] // P

                # phase A: gather x for level's parents; xT (kept per level);
                # wf = x @ W_f (psum) -> wf_lvl sbuf
                xlvl = lpool1.tile([P, max_nch_hi * P], bf, tag="xlvl", name=f"xl_{l}")
                for pc in range(nch):
                    gather_chunk(xlvl[:, pc * P : (pc + 1) * P], base_g + pc)
                xT_lvl = lpool.tile(
                    [P, max_nch_hi * P], bf, tag="xTlvl", name=f"xT_{l}"
                )
                for b0 in range(0, nch, gb):
                    w0 = min(gb, nch - b0)
                    nc.sync.dma_start_transpose(
                        xT_lvl[:, b0 * P : (b0 + w0) * P].rearrange(
                            "a (b c) -> a b c", b=w0
                        ),
                        xlvl[:, b0 * P : (b0 + w0) * P],
                    )
                wf_lvl = lpool.tile([P, max_nch_hi * P], bf, tag="wflvl", name=f"wf_{l}")
                for pc in range(nch):
                    g = base_g + pc
                    if allpad[g]:
                        continue
                    wf_ps = pszw.tile([P, P], f32, tag="zw", name=f"wfp_{l}_{pc}")
                    nc.tensor.matmul(
                        wf_ps[:],
                        xT_lvl[:, pc * P : (pc + 1) * P],
                        w_sb[:, 384:512],
                        start=True,
                        stop=True,
                    )
                    wfs = wf_lvl[:, pc * P : (pc + 1) * P]
                    if has_bias:
                        nc.vector.tensor_tensor(
                            wfs, wf_ps[:], bias_sb[:, 384:512], op=OP.add
                        )
                    else:
                        nc.vector.tensor_copy(wfs, wf_ps[:])

                # phase B: per edge chunk, f = sigmoid(h_ch @ U_f + wf[par]);
                # overwrite c (dead after its output DMA) with f*c in place.
                # chT via batched dma transpose over the interleaved c|h pairs.
                ec_pairs = {}
                for j, (ll, ec, pc) in enumerate(rel_cols):
                    if ll == l and not allpad[base_g + pc]:
                        ec_pairs.setdefault(ec, []).append((j, pc))
                for b0 in range(0, necs, gb):
                    w0 = min(gb, necs - b0)
                    cht = wpool2.tile(
                        [P, 2 * gb, P], bf, tag="cht", name=f"cht_{l}_{b0}"
                    )
                    in0 = 2 * (prev_base_g + b0) * P
                    nc.sync.dma_start_transpose(
                        cht[:, : 2 * w0, :],
                        hc_all[:, in0 : in0 + 2 * w0 * P],
                    )
                    for k in range(w0):
                        ec = b0 + k
                        gch = prev_base_g + ec
                        if allpad[gch]:
                            continue
                        prs = ec_pairs.get(ec, [])
                        z_ps = pszw.tile([P, P], f32, tag="zw", name=f"z_{l}_{ec}")
                        nc.tensor.matmul(
                            z_ps[:],
                            cht[:, 2 * k + 1, :],
                            uf_sb[:],
                            start=True,
                            stop=(len(prs) == 0),
                        )
                        for kk, (j, pc) in enumerate(prs):
                            nc.tensor.matmul(
                                z_ps[:],
                                oht_sb[:, j * P : (j + 1) * P],
                                wf_lvl[:, pc * P : (pc + 1) * P],
                                start=False,
                                stop=(kk == len(prs) - 1),
                            )
                        f_t = wpool.tile([P, P], bf, tag="ft", name=f"f_{l}_{ec}")
                        nc.scalar.activation(f_t[:], z_ps[:], AF.Sigmoid)
                        cc = cslice(gch)
                        nc.vector.tensor_tensor(cc, f_t[:], cc, op=OP.mult)

                # phase C: per parent chunk, segment sums + iou + gates
                by_pc = {}
                for ec, pc in pairs_by_level[l]:
                    by_pc.setdefault(pc, []).append(ec)
                for pc in range(nch):
                    g = base_g + pc
                    ecs = [e for e in by_pc.get(pc, []) if not allpad[prev_base_g + e]]
                    if allpad[g] or not ecs:
                        pad_chunk(g)
                        continue
                    segA = psA.tile([P, P], f32, tag="segA", name=f"sA_{l}_{pc}")
                    segB = psB.tile([P, P], f32, tag="segB", name=f"sB_{l}_{pc}")
                    for k, ec in enumerate(ecs):
                        gch = prev_base_g + ec
                        j = relcol_of[(l, ec, pc)]
                        oh_ap = oh_sb[:, j * P : (j + 1) * P]
                        fst, lst = k == 0, k == len(ecs) - 1
                        # h_sumT [H, par] accumulated directly: lhsT=ch
                        nc.tensor.matmul(
                            segA[:], hslice(gch), oh_ap, start=fst, stop=lst
                        )
                        nc.tensor.matmul(
                            segB[:], oh_ap, cslice(gch), start=fst, stop=lst
                        )
                    hsT = wpool.tile([P, P], bf, tag="hsT", name=f"hsT_{l}_{pc}")
                    nc.vector.tensor_copy(hsT[:], segA[:])
                    iou_ps = psio.tile([P, 384], f32, tag="psio", name=f"iou_{l}_{pc}")
                    nc.tensor.matmul(
                        iou_ps[:], hsT[:], uiou_sb[:], start=True, stop=False
                    )
                    nc.tensor.matmul(
                        iou_ps[:],
                        xT_lvl[:, pc * P : (pc + 1) * P],
                        w_sb[:, 0:384],
                        start=False,
                        stop=True,
                    )
                    if has_bias:
                        iou_sb = wpool.tile([P, 384], bf, tag="ioub", name=f"ib_{l}_{pc}")
                        nc.vector.tensor_tensor(
                            iou_sb[:], iou_ps[:], bias_sb[:, 0:384], op=OP.add
                        )
                        gates(g, iou_sb, segB[:])
                    else:
                        gates(g, iou_ps, segB[:])

    nc.finalize()
    return nc


# ---------------------------------------------------------------- entry point
def kernel(
    features,
    node_order,
    adjacency_list,
    edge_order,
    emb,
    W_iou,
    b_iou,
    U_iou,
    W_f,
    b_f,
    U_f,
    num_levels,
):
    import ml_dtypes
    from concourse.bass_utils import run_bass_kernel_spmd

    bf16 = ml_dtypes.bfloat16

    features = np.asarray(features)
    node_order = np.asarray(node_order)
    adjacency_list = np.asarray(adjacency_list)
    edge_order = np.asarray(edge_order)
    emb = np.asarray(emb, np.float32)
    W_iou = np.asarray(W_iou, np.float32)
    b_iou = np.asarray(b_iou, np.float32)
    U_iou = np.asarray(U_iou, np.float32)
    W_f = np.asarray(W_f, np.float32)
    b_f = np.asarray(b_f, np.float32)
    U_f = np.asarray(U_f, np.float32)
    L = int(num_levels)

    plan = build_plan(features, node_order, adjacency_list, edge_order, L)

    wcat = np.concatenate([W_iou, W_f], axis=1)
    bias = np.concatenate([b_iou, b_f])
    has_bias = bool(np.any(bias != 0.0))

    gb = int(os.environ.get("TREELSTM_GB", "4"))
    nc = build_bass(plan, int(emb.shape[0]), has_bias, gb=gb)

    NCH, NPAIR, NT = plan["NCH"], plan["NPAIR"], plan["NT"]
    emb_bf = np.ascontiguousarray(emb.astype(bf16))
    wcat_bf = np.ascontiguousarray(wcat.astype(bf16))
    uiou_bf = np.ascontiguousarray(U_iou.astype(bf16))
    uf_bf = np.ascontiguousarray(U_f.astype(bf16))

    ar = np.arange(P, dtype=np.float32)
    in_maps = []
    for c in range(NCORES):
        m = {
            "emb": emb_bf,
            "wcat": wcat_bf,
            "uiou": uiou_bf,
            "uf": uf_bf,
            "featidx": np.ascontiguousarray(plan["featidx"][c].reshape(NCH, P).T),
            "maskv": np.ascontiguousarray(
                plan["maskv"][c].reshape(NCH, P).T.astype(bf16)
            ),
        }
        if NPAIR:
            oh = plan["rel"][c][:, :, None] == ar[None, None, :]  # [J, e, p]
            m["ohcat"] = np.ascontiguousarray(
                oh.transpose(1, 0, 2).reshape(P, NPAIR * P).astype(bf16)
            )
            m["ohtcat"] = np.ascontiguousarray(
                oh.transpose(2, 0, 1).reshape(P, NPAIR * P).astype(bf16)
            )
        if has_bias:
            m["bias"] = np.ascontiguousarray(
                np.broadcast_to(bias[None, :], (P, 512)).astype(bf16)
            )
        in_maps.append(m)

    trace = os.environ.get("TREELSTM_TRACE", "0") == "1"
    res = run_bass_kernel_spmd(nc, in_maps, list(range(NCORES)), trace=trace)
    if trace and res.exec_time_ns is not None:
        print(f"HW exec time: {res.exec_time_ns} ns", flush=True)

    N = plan["N"]
    h_full = np.zeros((N, P), np.float32)
    c_full = np.zeros((N, P), np.float32)
    for c in range(NCORES):
        gid = plan["gids"][c]
        rows = np.flatnonzero(gid >= 0)
        hc = res.results[c]["outhc"].reshape(NT, 2, P).astype(np.float32)
        c_full[gid[rows]] = hc[rows, 0]
        h_full[gid[rows]] = hc[rows, 1]
    return h_full, c_full
